# revision 1
# baseline (speedup 1.0000x reference)
# MoE layer (B=4,S=2048,D=1024,F=4096,E=8,top-2) on 8 Trainium2 NeuronCores.
#
# Strategy: the gate (softmax + top-2, ~8192x8) is computed on the host in
# fp32 — it is 0.005% of the FLOPs. The heavy per-expert FFN GEMMs run on
# device. Each (expert, F-quarter) is a job; each expert's routed tokens are
# padded to 256-token chunks; a planner packs job pieces into an identical
# per-core group schedule (same NEFF on all 8 cores, different data).
# Per chunk the device computes, in transposed activation layout:
#   h^T = gelu(W1q^T @ X^T + b1)          (F-quarter on partitions)
#   y^T = (W2q^T @ h^T + b2q) * combine   (D on partitions)
# with all matmuls in float32r (tf32-class precision, bf16-rate). The host
# scatter-adds the 4 quarter partials of each token-expert pair into the
# full output. Zero-padded chunk slots produce combine=0 rows that are
# dropped on the host.
import sys

for _p in ("/opt/trn_rl_repo",):
    if _p not in sys.path:
        sys.path.insert(0, _p)

import numpy as np
from contextlib import ExitStack

B, S, D, F, E, TOPK = 4, 2048, 1024, 4096, 8, 2
LBW = 0.01
T = B * S

NCORES = 8
NT = 256            # tokens per chunk (matmul moving dim)
QUARTERS = 4        # F split across independent jobs
FQ = F // QUARTERS  # 1024
KD = D // 128       # 8  k-subtiles for mm1
FTILES = FQ // 128  # 8  output tiles of mm1 / k-subtiles of mm2
DTILES = D // 128   # 8  output tiles of mm2


# ---------------------------------------------------------------- routing --
def _route(x, expert_weights, gate_w, gate_b):
    """Host fp32 gate: returns combine (T,E), load_loss (np.float32)."""
    xf = x.reshape(-1, D)
    logits = xf @ gate_w + gate_b                      # (T, E) fp32
    z = logits + np.repeat(expert_weights, S, axis=0)  # (T, E)
    m = z.max(axis=-1, keepdims=True)
    p = np.exp(z - m, dtype=np.float32)
    p /= p.sum(axis=-1, keepdims=True)                 # gate_probs fp32

    order = np.argsort(-p, axis=-1, kind="stable")     # ties -> lower index
    topi = order[:, :TOPK]                             # (T, K)
    topp = np.take_along_axis(p, topi, axis=-1)
    topp = topp / topp.sum(axis=-1, keepdims=True)

    combine = np.zeros((T, E), np.float32)
    np.put_along_axis(combine, topi, topp, axis=-1)

    top1 = np.argmax(p, axis=-1)
    f_frac = np.bincount(top1, minlength=E).astype(np.float32) / np.float32(T)
    P_mean = p.mean(axis=0, dtype=np.float32)
    load_loss = np.float32(LBW * E * np.sum(f_frac * P_mean, dtype=np.float32))
    return combine, load_loss


# ---------------------------------------------------------------- planner --
def _plan(chunk_counts):
    """chunk_counts[e] = number of NT-token chunks routed to expert e.

    Jobs are (expert, quarter) with that chunk count. Returns
    (caps, assign): caps[g] = chunks in group g (same on every core);
    assign[g][core] = (e, q, chunk_off, n) or None.
    Greedy: each round serves the 8 largest jobs with a group sized to the
    8th-largest (exact packing for balanced inputs)."""
    jobs = [[chunk_counts[e], e, q, 0]
            for e in range(E) for q in range(QUARTERS) if chunk_counts[e] > 0]
    caps, assign = [], []
    while jobs:
        jobs.sort(key=lambda j: -j[0])
        take = jobs[:NCORES]
        if len(jobs) >= NCORES:
            s = take[-1][0]
        else:
            s = max(j[0] for j in take)
        s = max(s, 1)
        caps.append(s)
        row = [None] * NCORES
        for i, job in enumerate(take):
            n = min(job[0], s)
            row[i] = (job[1], job[2], job[3], n)
            job[0] -= n
            job[3] += n
        assign.append(row)
        jobs = [j for j in jobs if j[0] > 0]
    return caps, assign


# ------------------------------------------------------------ bass kernel --
def _build(caps):
    import concourse.bacc as bacc
    import concourse.tile as tile
    import concourse.mybir as mybir

    F32 = mybir.dt.float32
    F32R = mybir.dt.float32r
    AF = mybir.ActivationFunctionType
    ALU = mybir.AluOpType

    G = len(caps)
    TOTCH = sum(caps)

    nc = bacc.Bacc("TRN2", debug=False, num_devices=NCORES)
    xt = nc.dram_tensor("xt", [TOTCH, 128, KD, NT], F32R, kind="ExternalInput")
    w1 = nc.dram_tensor("w1", [G, 128, KD, FQ], F32R, kind="ExternalInput")
    w2 = nc.dram_tensor("w2", [G, 128, FTILES, D], F32R, kind="ExternalInput")
    b1 = nc.dram_tensor("b1", [G, 128, FTILES], F32, kind="ExternalInput")
    b2 = nc.dram_tensor("b2", [G, 128, DTILES], F32, kind="ExternalInput")
    comb = nc.dram_tensor("comb", [TOTCH, 128, NT], F32, kind="ExternalInput")
    y = nc.dram_tensor("y", [TOTCH, 128, DTILES, NT], F32, kind="ExternalOutput")

    with tile.TileContext(nc) as tc, ExitStack() as ctx:
        wpool = ctx.enter_context(tc.tile_pool(name="w", bufs=2))
        bpool = ctx.enter_context(tc.tile_pool(name="b", bufs=2))
        xpool = ctx.enter_context(tc.tile_pool(name="x", bufs=4))
        hpool = ctx.enter_context(tc.tile_pool(name="h", bufs=4))
        ypool = ctx.enter_context(tc.tile_pool(name="y", bufs=3))
        pph = ctx.enter_context(tc.tile_pool(name="ph", bufs=2, space="PSUM"))
        ppo = ctx.enter_context(tc.tile_pool(name="po", bufs=1, space="PSUM"))

        slot = 0
        for g in range(G):
            w1t = wpool.tile([128, KD, FQ], F32R, tag="w1")
            nc.gpsimd.dma_start(w1t[:], w1[g])
            w2t = wpool.tile([128, FTILES, D], F32R, tag="w2")
            nc.gpsimd.dma_start(w2t[:], w2[g])
            b1t = bpool.tile([128, FTILES], F32, tag="b1")
            nc.gpsimd.dma_start(b1t[:], b1[g])
            b2t = bpool.tile([128, DTILES], F32, tag="b2")
            nc.gpsimd.dma_start(b2t[:], b2[g])

            for _ch in range(caps[g]):
                xt_t = xpool.tile([128, KD, NT], F32R, tag="xt")
                nc.sync.dma_start(xt_t[:], xt[slot])
                cb_t = xpool.tile([128, NT], F32, tag="cb")
                nc.sync.dma_start(cb_t[:], comb[slot])

                po = ppo.tile([128, DTILES, NT], F32, tag="po")
                hbs = []

                def mm2(f):
                    for d in range(DTILES):
                        nc.tensor.matmul(
                            po[:, d, :],
                            lhsT=w2t[:, f, d * 128:(d + 1) * 128],
                            rhs=hbs[f][:],
                            start=(f == 0 and d % 2 == 0),
                            stop=(f == FTILES - 1 and d % 2 == 1),
                        )

                for f in range(FTILES):
                    ph = pph.tile([128, NT], F32, tag="ph")
                    for k in range(KD):
                        nc.tensor.matmul(
                            ph[:],
                            lhsT=w1t[:, k, f * 128:(f + 1) * 128],
                            rhs=xt_t[:, k, :],
                            start=(k == 0),
                            stop=(k == KD - 1),
                        )
                    hb = hpool.tile([128, NT], F32R, tag="hb")
                    nc.scalar.activation(hb[:], ph[:], AF.Gelu, bias=b1t[:, f:f + 1])
                    hbs.append(hb)
                    if f > 0:
                        mm2(f - 1)
                mm2(FTILES - 1)

                yt = ypool.tile([128, DTILES, NT], F32, tag="yt")
                nc.vector.tensor_tensor(
                    yt[:], po[:],
                    b2t[:, :, None].to_broadcast([128, DTILES, NT]), ALU.add)
                nc.vector.tensor_tensor(
                    yt[:], yt[:],
                    cb_t[:, None, :].to_broadcast([128, DTILES, NT]), ALU.mult)
                nc.sync.dma_start(y[slot], yt[:])
                slot += 1
    nc.compile()
    return nc


# ------------------------------------------------------------------ main --
LAST_EXEC_NS = None


def kernel(x, expert_weights, gate_w, gate_b, w1, b1, w2, b2, _trace=False):
    global LAST_EXEC_NS
    from concourse.bass_utils import run_bass_kernel_spmd

    x = np.asarray(x, np.float32)
    expert_weights = np.asarray(expert_weights, np.float32)
    gate_w = np.asarray(gate_w, np.float32)
    gate_b = np.asarray(gate_b, np.float32)
    w1 = np.asarray(w1, np.float32)
    b1 = np.asarray(b1, np.float32)
    w2 = np.asarray(w2, np.float32)
    b2 = np.asarray(b2, np.float32)

    combine, load_loss = _route(x, expert_weights, gate_w, gate_b)

    # token lists per expert, padded to NT multiples (pad index = -1)
    idx_e, pad_idx_e, chunk_counts = [], [], []
    for e in range(E):
        idx = np.nonzero(combine[:, e] > 0)[0].astype(np.int64)
        nch = -(-len(idx) // NT) if len(idx) else 0
        padded = np.full(nch * NT, -1, np.int64)
        padded[:len(idx)] = idx
        idx_e.append(idx)
        pad_idx_e.append(padded)
        chunk_counts.append(nch)

    caps, assign = _plan(chunk_counts)
    G, TOTCH = len(caps), sum(caps)
    gbase = np.cumsum([0] + caps)

    # ---- host-side input assembly --------------------------------------
    xf = x.reshape(T, D)
    X_T = np.ascontiguousarray(xf.T)  # (D, T)

    w1c, w2c, b1c = {}, {}, {}
    for e in range(E):
        for q in range(QUARTERS):
            pass  # filled lazily below

    def w1_block(e, q):
        key = (e, q)
        if key not in w1c:
            blk = w1[e][:, q * FQ:(q + 1) * FQ]            # (D, FQ)
            w1c[key] = np.ascontiguousarray(
                blk.reshape(KD, 128, FQ).transpose(1, 0, 2))
        return w1c[key]

    def w2_block(e, q):
        key = (e, q)
        if key not in w2c:
            blk = w2[e][q * FQ:(q + 1) * FQ, :]            # (FQ, D)
            w2c[key] = np.ascontiguousarray(
                blk.reshape(FTILES, 128, D).transpose(1, 0, 2))
        return w2c[key]

    def b1_block(e, q):
        key = (e, q)
        if key not in b1c:
            b1c[key] = np.ascontiguousarray(
                b1[e][q * FQ:(q + 1) * FQ].reshape(FTILES, 128).T)
        return b1c[key]

    in_maps = []
    piece_meta = []  # per core: list of (slot0, n, e, tokens)
    for core in range(NCORES):
        xt_in = np.zeros((TOTCH, 128, KD, NT), np.float32)
        cb_in = np.zeros((TOTCH, 128, NT), np.float32)
        w1_in = np.zeros((G, 128, KD, FQ), np.float32)
        w2_in = np.zeros((G, 128, FTILES, D), np.float32)
        b1_in = np.zeros((G, 128, FTILES), np.float32)
        b2_in = np.zeros((G, 128, DTILES), np.float32)
        metas = []
        for g in range(G):
            piece = assign[g][core]
            if piece is None:
                continue
            e, q, off, n = piece
            w1_in[g] = w1_block(e, q)
            w2_in[g] = w2_block(e, q)
            b1_in[g] = b1_block(e, q)
            if q == 0:
                b2_in[g] = b2[e].reshape(DTILES, 128).T
            toks = pad_idx_e[e][off * NT:(off + n) * NT]
            valid = toks >= 0
            seg = np.zeros((D, n * NT), np.float32)
            seg[:, valid] = X_T[:, toks[valid]]
            s0 = gbase[g]
            xt_in[s0:s0 + n] = seg.reshape(KD, 128, n, NT).transpose(2, 1, 0, 3)
            cvals = np.zeros(n * NT, np.float32)
            cvals[valid] = combine[toks[valid], e]
            cb_in[s0:s0 + n] = cvals.reshape(n, 1, NT)
            metas.append((s0, n, e, toks))
        piece_meta.append(metas)
        in_maps.append({"xt": xt_in, "w1": w1_in, "w2": w2_in,
                        "b1": b1_in, "b2": b2_in, "comb": cb_in})

    nc = _build(caps)
    res = run_bass_kernel_spmd(nc, in_maps, list(range(NCORES)), trace=_trace)
    LAST_EXEC_NS = res.exec_time_ns

    # ---- gather / unshard ----------------------------------------------
    out = np.zeros((T, D), np.float32)
    for core in range(NCORES):
        yc = res.results[core]["y"]  # (TOTCH, 128, DTILES, NT)
        for (s0, n, e, toks) in piece_meta[core]:
            blk = yc[s0:s0 + n]                       # (n, 128, DTILES, NT)
            cols = blk.transpose(2, 1, 0, 3).reshape(D, n * NT)
            valid = toks >= 0
            out[toks[valid]] += cols[:, valid].T
    return out.reshape(B, S, D), load_loss


# revision 5
# speedup vs baseline: 1.0429x; 1.0429x over previous
# MoE layer (B=4,S=2048,D=1024,F=4096,E=8,top-2) on 8 Trainium2 NeuronCores.
#
# Strategy: the gate (softmax + top-2, ~8192x8) is computed on the host in
# fp32 — it is 0.005% of the FLOPs. The heavy per-expert FFN GEMMs run on
# device. Each (expert, F-quarter) is a job; each expert's routed tokens are
# padded to 256-token chunks; a planner packs job pieces into an identical
# per-core group schedule (same NEFF on all 8 cores, different data).
#
# Device math per 512-token chunk-pair (float32r matmuls, tf32-class):
#   mm1: h^T[f,:] = gelu(W1q[:,f]^T @ X^T + b1[f])   f = 8 tiles of FQ=1024
#        (lhsT = W1q k-subtile, moving = X^T, N=512 tokens)
#   mm2: y[toks,:] += hb[f][:,ttile]^T @ W2q[f,:]    (lhsT = h^T token block,
#        moving = W2q, N=512 of D; tokens land on PSUM partitions)
#   out: y = (y + b2q) * combine  — b2 via DVE add, combine via ACT
#        per-partition scale; output is token-major so the host gather is a
#        plain row scatter-add.
# The host scatter-adds the 4 F-quarter partials of each token-expert pair
# into the full output. Zero-padded slots have combine=0 and are dropped.
import sys

for _p in ("/opt/trn_rl_repo",):
    if _p not in sys.path:
        sys.path.insert(0, _p)

import numpy as np
from contextlib import ExitStack

B, S, D, F, E, TOPK = 4, 2048, 1024, 4096, 8, 2
LBW = 0.01
T = B * S

NCORES = 8
NT = 256            # tokens per chunk (scheduling granularity)
QUARTERS = 4        # F split across independent jobs
FQ = F // QUARTERS  # 1024
KD = D // 128       # 8  k-subtiles for mm1
FTILES = FQ // 128  # 8  h tiles / mm2 contraction subtiles
DCH = 512           # mm2 moving chunk of D
NDC = D // DCH      # 2


# ---------------------------------------------------------------- routing --
def _route(x, expert_weights, gate_w, gate_b):
    """Host fp32 gate: returns combine (T,E), load_loss (np.float32)."""
    xf = x.reshape(-1, D)
    logits = xf @ gate_w + gate_b                      # (T, E) fp32
    z = logits + np.repeat(expert_weights, S, axis=0)  # (T, E)
    m = z.max(axis=-1, keepdims=True)
    p = np.exp(z - m, dtype=np.float32)
    p /= p.sum(axis=-1, keepdims=True)                 # gate_probs fp32

    order = np.argsort(-p, axis=-1, kind="stable")     # ties -> lower index
    topi = order[:, :TOPK]                             # (T, K)
    topp = np.take_along_axis(p, topi, axis=-1)
    topp = topp / topp.sum(axis=-1, keepdims=True)

    combine = np.zeros((T, E), np.float32)
    np.put_along_axis(combine, topi, topp, axis=-1)

    top1 = np.argmax(p, axis=-1)
    f_frac = np.bincount(top1, minlength=E).astype(np.float32) / np.float32(T)
    P_mean = p.mean(axis=0, dtype=np.float32)
    load_loss = np.float32(LBW * E * np.sum(f_frac * P_mean, dtype=np.float32))
    return combine, load_loss


# ---------------------------------------------------------------- planner --
def _plan(chunk_counts):
    """chunk_counts[e] = number of NT-token chunks routed to expert e.

    Jobs are (expert, quarter) with that chunk count. Returns
    (caps, assign): caps[g] = chunks in group g (same on every core);
    assign[g][core] = (e, q, chunk_off, n) or None.
    Greedy: each round serves the 8 largest jobs with a group sized to the
    8th-largest (exact packing for balanced inputs)."""
    jobs = [[chunk_counts[e], e, q, 0]
            for e in range(E) for q in range(QUARTERS) if chunk_counts[e] > 0]
    caps, assign = [], []
    while jobs:
        jobs.sort(key=lambda j: -j[0])
        take = jobs[:NCORES]
        if len(jobs) >= NCORES:
            s = take[-1][0]
        else:
            s = max(j[0] for j in take)
        s = max(s, 1)
        caps.append(s)
        row = [None] * NCORES
        for i, job in enumerate(take):
            n = min(job[0], s)
            row[i] = (job[1], job[2], job[3], n)
            job[0] -= n
            job[3] += n
        assign.append(row)
        jobs = [j for j in jobs if j[0] > 0]
    return caps, assign


# ------------------------------------------------------------ bass kernel --
def _build(caps):
    import concourse.bacc as bacc
    import concourse.tile as tile
    import concourse.mybir as mybir

    F32 = mybir.dt.float32
    F32R = mybir.dt.float32r
    AF = mybir.ActivationFunctionType
    ALU = mybir.AluOpType

    G = len(caps)
    TOTCH = sum(caps)

    nc = bacc.Bacc("TRN2", debug=False, num_devices=NCORES)
    xt = nc.dram_tensor("xt", [TOTCH, 128, KD, NT], F32R, kind="ExternalInput")
    w1 = nc.dram_tensor("w1", [G, FTILES, 128, KD, 128], F32R, kind="ExternalInput")
    w2 = nc.dram_tensor("w2", [G, FTILES, 128, D], F32R, kind="ExternalInput")
    b1 = nc.dram_tensor("b1", [G, 128, FTILES], F32, kind="ExternalInput")
    b2 = nc.dram_tensor("b2", [G, 128, D], F32, kind="ExternalInput")
    comb = nc.dram_tensor("comb", [TOTCH, 128, 2], F32, kind="ExternalInput")
    y = nc.dram_tensor("y", [TOTCH, NT, D], F32, kind="ExternalOutput")

    with tile.TileContext(nc) as tc, ExitStack() as ctx:
        w1pool = ctx.enter_context(tc.tile_pool(name="w1p", bufs=12))
        w2pool = ctx.enter_context(tc.tile_pool(name="w2p", bufs=12))
        bpool = ctx.enter_context(tc.tile_pool(name="bp", bufs=2))
        xpool = ctx.enter_context(tc.tile_pool(name="xp", bufs=2))
        cpool = ctx.enter_context(tc.tile_pool(name="cp", bufs=4))
        hpool = ctx.enter_context(tc.tile_pool(name="hp", bufs=17))
        ypool = ctx.enter_context(tc.tile_pool(name="yp", bufs=3))
        pph = ctx.enter_context(tc.tile_pool(name="ph", bufs=2, space="PSUM"))
        ppo = ctx.enter_context(tc.tile_pool(name="po", bufs=3, space="PSUM"))

        # build the unit list: (slot, ntok) with ntok 512 (pair) or 256
        units = []
        slot = 0
        for g in range(G):
            n = caps[g]
            for _ in range(n // 2):
                units.append((g, slot, 2))
                slot += 2
            if n % 2:
                units.append((g, slot, 1))
                slot += 1

        state = {}   # per emitted unit: (ntok, hbs, cb, w2 tiles, b2, slot0)
        pending = None

        def emit_mm2(u):
            ntok, hbs, cb_t, w2f_l, b2t_l, slot0 = state[u]
            for t in range(2 * ntok):
                po = ppo.tile([128, D], F32, tag="po")
                for f in range(FTILES):
                    for dc in range(NDC):
                        nc.tensor.matmul(
                            po[:, dc * DCH:(dc + 1) * DCH],
                            lhsT=hbs[f][:, t * 128:(t + 1) * 128],
                            rhs=w2f_l[f][:, dc * DCH:(dc + 1) * DCH],
                            start=(f == 0),
                            stop=(f == FTILES - 1),
                        )
                yt = ypool.tile([128, D], F32, tag="yt")
                nc.vector.tensor_tensor(yt[:], po[:], b2t_l[:], ALU.add)
                yt2 = ypool.tile([128, D], F32, tag="yt2")
                nc.scalar.activation(yt2[:], yt[:], AF.Identity,
                                     scale=cb_t[:, t:t + 1])
                sl, tt = divmod(t, 2)
                nc.scalar.dma_start(
                    y[slot0 + sl, tt * 128:(tt + 1) * 128, :], yt2[:])

        gcur = -1
        w1f = w2f = b1t = b2t = None
        for ui, (g, slot0, ntok) in enumerate(units):
            if g != gcur:
                gcur = g
                b1t = bpool.tile([128, FTILES], F32, tag="b1")
                nc.gpsimd.dma_start(b1t[:], b1[g])
                w1f = []
                for f in range(FTILES):
                    wt = w1pool.tile([128, KD, 128], F32R, tag="w1f")
                    nc.gpsimd.dma_start(wt[:], w1[g, f])
                    w1f.append(wt)
                w2f = []
                for f in range(FTILES):
                    wt = w2pool.tile([128, D], F32R, tag="w2f")
                    nc.gpsimd.dma_start(wt[:], w2[g, f])
                    w2f.append(wt)
                b2t = bpool.tile([128, D], F32, tag="b2")
                nc.gpsimd.dma_start(b2t[:], b2[g])

            ntok_el = ntok * NT
            xt_t = xpool.tile([128, KD, 2 * NT], F32R, tag="xt")
            for s in range(ntok):
                nc.sync.dma_start(xt_t[:, :, s * NT:(s + 1) * NT], xt[slot0 + s])
            cb_t = cpool.tile([128, 4], F32, tag="cb")
            for s in range(ntok):
                nc.sync.dma_start(cb_t[:, 2 * s:2 * s + 2], comb[slot0 + s])

            hbs = []
            for f in range(FTILES):
                ph = pph.tile([128, 2 * NT], F32, tag="ph")
                for k in range(KD):
                    nc.tensor.matmul(
                        ph[:, :ntok_el],
                        lhsT=w1f[f][:, k, :],
                        rhs=xt_t[:, k, :ntok_el],
                        start=(k == 0),
                        stop=(k == KD - 1),
                    )
                hb = hpool.tile([128, 2 * NT], F32R, tag="hb")
                nc.scalar.activation(hb[:, :ntok_el], ph[:, :ntok_el],
                                     AF.Gelu, bias=b1t[:, f:f + 1])
                hbs.append(hb)
            state[ui] = (ntok, hbs, cb_t, list(w2f), b2t, slot0)
            if pending is not None:
                emit_mm2(pending)
                del state[pending]
            pending = ui
        if pending is not None:
            emit_mm2(pending)
    nc.compile()
    return nc


# ------------------------------------------------------------------ main --
LAST_EXEC_NS = None


def kernel(x, expert_weights, gate_w, gate_b, w1, b1, w2, b2, _trace=False):
    global LAST_EXEC_NS
    from concourse.bass_utils import run_bass_kernel_spmd

    x = np.asarray(x, np.float32)
    expert_weights = np.asarray(expert_weights, np.float32)
    gate_w = np.asarray(gate_w, np.float32)
    gate_b = np.asarray(gate_b, np.float32)
    w1 = np.asarray(w1, np.float32)
    b1 = np.asarray(b1, np.float32)
    w2 = np.asarray(w2, np.float32)
    b2 = np.asarray(b2, np.float32)

    combine, load_loss = _route(x, expert_weights, gate_w, gate_b)

    # token lists per expert, padded to NT multiples (pad index = -1)
    pad_idx_e, chunk_counts = [], []
    for e in range(E):
        idx = np.nonzero(combine[:, e] > 0)[0].astype(np.int64)
        nch = -(-len(idx) // NT) if len(idx) else 0
        padded = np.full(nch * NT, -1, np.int64)
        padded[:len(idx)] = idx
        pad_idx_e.append(padded)
        chunk_counts.append(nch)

    caps, assign = _plan(chunk_counts)
    G, TOTCH = len(caps), sum(caps)
    gbase = np.cumsum([0] + caps)

    # ---- host-side input assembly --------------------------------------
    xf = x.reshape(T, D)
    X_T = np.ascontiguousarray(xf.T)  # (D, T)

    w1c, w2c, b1c = {}, {}, {}

    def w1_block(e, q):
        key = (e, q)
        if key not in w1c:
            blk = w1[e][:, q * FQ:(q + 1) * FQ]            # (D, FQ)
            # [f, p, k, m] = W1q[k*128+p, f*128+m]
            w1c[key] = np.ascontiguousarray(
                blk.reshape(KD, 128, FTILES, 128).transpose(2, 1, 0, 3))
        return w1c[key]

    def w2_block(e, q):
        key = (e, q)
        if key not in w2c:
            blk = w2[e][q * FQ:(q + 1) * FQ, :]            # (FQ, D)
            # [f, p, :] = W2q[f*128+p, :]
            w2c[key] = np.ascontiguousarray(blk.reshape(FTILES, 128, D))
        return w2c[key]

    def b1_block(e, q):
        key = (e, q)
        if key not in b1c:
            b1c[key] = np.ascontiguousarray(
                b1[e][q * FQ:(q + 1) * FQ].reshape(FTILES, 128).T)
        return b1c[key]

    in_maps = []
    piece_meta = []  # per core: list of (slot0, n, toks)
    for core in range(NCORES):
        xt_in = np.zeros((TOTCH, 128, KD, NT), np.float32)
        cb_in = np.zeros((TOTCH, 128, 2), np.float32)
        w1_in = np.zeros((G, FTILES, 128, KD, 128), np.float32)
        w2_in = np.zeros((G, FTILES, 128, D), np.float32)
        b1_in = np.zeros((G, 128, FTILES), np.float32)
        b2_in = np.zeros((G, 128, D), np.float32)
        metas = []
        for g in range(G):
            piece = assign[g][core]
            if piece is None:
                continue
            e, q, off, n = piece
            w1_in[g] = w1_block(e, q)
            w2_in[g] = w2_block(e, q)
            b1_in[g] = b1_block(e, q)
            if q == 0:
                b2_in[g] = b2[e][None, :]
            toks = pad_idx_e[e][off * NT:(off + n) * NT]
            valid = toks >= 0
            seg = np.zeros((D, n * NT), np.float32)
            seg[:, valid] = X_T[:, toks[valid]]
            s0 = gbase[g]
            xt_in[s0:s0 + n] = seg.reshape(KD, 128, n, NT).transpose(2, 1, 0, 3)
            cvals = np.zeros(n * NT, np.float32)
            cvals[valid] = combine[toks[valid], e]
            cb_in[s0:s0 + n] = cvals.reshape(n, 2, 128).transpose(0, 2, 1)
            metas.append((s0, n, toks))
        piece_meta.append(metas)
        in_maps.append({"xt": xt_in, "w1": w1_in, "w2": w2_in,
                        "b1": b1_in, "b2": b2_in, "comb": cb_in})

    nc = _build(caps)
    res = run_bass_kernel_spmd(nc, in_maps, list(range(NCORES)), trace=_trace)
    LAST_EXEC_NS = res.exec_time_ns

    # ---- gather / unshard ----------------------------------------------
    out = np.zeros((T, D), np.float32)
    for core in range(NCORES):
        yc = res.results[core]["y"]  # (TOTCH, NT, D), token-major
        for (s0, n, toks) in piece_meta[core]:
            rows = yc[s0:s0 + n].reshape(n * NT, D)
            valid = toks >= 0
            out[toks[valid]] += rows[valid]
    return out.reshape(B, S, D), load_loss


# revision 12
# speedup vs baseline: 1.0644x; 1.0206x over previous
# MoE layer (B=4,S=2048,D=1024,F=4096,E=8,top-2) on 8 Trainium2 NeuronCores.
#
# Strategy: the gate (softmax + top-2, ~8192x8) is computed on the host in
# fp32 — it is 0.005% of the FLOPs. The heavy per-expert FFN GEMMs run on
# device. Each (expert, F-quarter) is a job; each expert's routed tokens are
# padded to 256-token chunks; a planner packs job pieces into an identical
# per-core group schedule (same NEFF on all 8 cores, different data).
#
# Device math per 512-token chunk-pair (float32r matmuls, tf32-class):
#   mm1: h^T[f,:] = gelu(W1q[:,f]^T @ X^T + b1[f])   f = 8 tiles of FQ=1024
#        (lhsT = W1q k-subtile, moving = X^T, N=512 tokens)
#   mm2: y[toks,:] += hb[f][:,ttile]^T @ W2q[f,:]    (lhsT = h^T token block,
#        moving = W2q, N=512 of D; tokens land on PSUM partitions)
#   out: y = (y + b2q) * combine  — b2 via DVE add, combine via ACT
#        per-partition scale; output is token-major so the host gather is a
#        plain row scatter-add.
# The host scatter-adds the 4 F-quarter partials of each token-expert pair
# into the full output. Zero-padded slots have combine=0 and are dropped.
import sys

for _p in ("/opt/trn_rl_repo",):
    if _p not in sys.path:
        sys.path.insert(0, _p)

import numpy as np
from contextlib import ExitStack

B, S, D, F, E, TOPK = 4, 2048, 1024, 4096, 8, 2
LBW = 0.01
T = B * S

NCORES = 8
NT = 256            # tokens per chunk (scheduling granularity)
QUARTERS = 4        # F split across independent jobs
FQ = F // QUARTERS  # 1024
KD = D // 128       # 8  k-subtiles for mm1
FTILES = FQ // 128  # 8  h tiles / mm2 contraction subtiles
DCH = 512           # mm2 moving chunk of D
NDC = D // DCH      # 2


def _units(caps):
    """Chunk-pair work units: list of (group, first_chunk_slot, ntok_chunks)."""
    units, slot = [], 0
    for g, n in enumerate(caps):
        for _ in range(n // 2):
            units.append((g, slot, 2))
            slot += 2
        if n % 2:
            units.append((g, slot, 1))
            slot += 1
    return units


# ---------------------------------------------------------------- routing --
def _route(x, expert_weights, gate_w, gate_b):
    """Host fp32 gate: returns combine (T,E), load_loss (np.float32)."""
    xf = x.reshape(-1, D)
    logits = xf @ gate_w + gate_b                      # (T, E) fp32
    z = logits + np.repeat(expert_weights, S, axis=0)  # (T, E)
    m = z.max(axis=-1, keepdims=True)
    p = np.exp(z - m, dtype=np.float32)
    p /= p.sum(axis=-1, keepdims=True)                 # gate_probs fp32

    order = np.argsort(-p, axis=-1, kind="stable")     # ties -> lower index
    topi = order[:, :TOPK]                             # (T, K)
    topp = np.take_along_axis(p, topi, axis=-1)
    topp = topp / topp.sum(axis=-1, keepdims=True)

    combine = np.zeros((T, E), np.float32)
    np.put_along_axis(combine, topi, topp, axis=-1)

    top1 = np.argmax(p, axis=-1)
    f_frac = np.bincount(top1, minlength=E).astype(np.float32) / np.float32(T)
    P_mean = p.mean(axis=0, dtype=np.float32)
    load_loss = np.float32(LBW * E * np.sum(f_frac * P_mean, dtype=np.float32))
    return combine, load_loss


# ---------------------------------------------------------------- planner --
def _plan(chunk_counts):
    """chunk_counts[e] = number of NT-token chunks routed to expert e.

    Jobs are (expert, quarter) with that chunk count. Returns
    (caps, assign): caps[g] = chunks in group g (same on every core);
    assign[g][core] = (e, q, chunk_off, n) or None.
    Greedy: each round serves the 8 largest jobs with a group sized to the
    8th-largest (exact packing for balanced inputs)."""
    jobs = [[chunk_counts[e], e, q, 0]
            for e in range(E) for q in range(QUARTERS) if chunk_counts[e] > 0]
    caps, assign = [], []
    while jobs:
        jobs.sort(key=lambda j: -j[0])
        take = jobs[:NCORES]
        if len(jobs) >= NCORES:
            s = take[-1][0]
        else:
            s = max(j[0] for j in take)
        s = max(s, 1)
        caps.append(s)
        row = [None] * NCORES
        for i, job in enumerate(take):
            n = min(job[0], s)
            row[i] = (job[1], job[2], job[3], n)
            job[0] -= n
            job[3] += n
        assign.append(row)
        jobs = [j for j in jobs if j[0] > 0]
    return caps, assign


# ------------------------------------------------------------ bass kernel --
def _build(caps):
    import concourse.bacc as bacc
    import concourse.tile as tile
    import concourse.mybir as mybir

    F32 = mybir.dt.float32
    F32R = mybir.dt.float32r
    AF = mybir.ActivationFunctionType
    ALU = mybir.AluOpType

    G = len(caps)
    TOTCH = sum(caps)

    units = _units(caps)
    NU = len(units)

    nc = bacc.Bacc("TRN2", debug=False, num_devices=NCORES)
    xt = nc.dram_tensor("xt", [NU, KD, 128, 2 * NT], F32R, kind="ExternalInput")
    w1 = nc.dram_tensor("w1", [G, FTILES, 128, KD, 128], F32R, kind="ExternalInput")
    w2 = nc.dram_tensor("w2", [G, FTILES, 128, D], F32R, kind="ExternalInput")
    b1 = nc.dram_tensor("b1", [G, 128, FTILES], F32, kind="ExternalInput")
    b2 = nc.dram_tensor("b2", [G, 128, D], F32, kind="ExternalInput")
    comb = nc.dram_tensor("comb", [NU, 128, 4], F32, kind="ExternalInput")
    y = nc.dram_tensor("y", [TOTCH, NT, D], F32, kind="ExternalOutput")

    with tile.TileContext(nc) as tc, ExitStack() as ctx:
        w1pool = ctx.enter_context(tc.tile_pool(name="w1p", bufs=12))
        w2pool = ctx.enter_context(tc.tile_pool(name="w2p", bufs=12))
        bpool = ctx.enter_context(tc.tile_pool(name="bp", bufs=2))
        xpool = ctx.enter_context(tc.tile_pool(name="xp", bufs=18))
        cpool = ctx.enter_context(tc.tile_pool(name="cp", bufs=4))
        hpool = ctx.enter_context(tc.tile_pool(name="hp", bufs=17))
        ypool = ctx.enter_context(tc.tile_pool(name="yp", bufs=3))
        pph = ctx.enter_context(tc.tile_pool(name="ph", bufs=2, space="PSUM"))
        ppo = ctx.enter_context(tc.tile_pool(name="po", bufs=3, space="PSUM"))

        state = {}   # per emitted unit: (ntok, hbs, cb, w2 tiles, b2, slot0)
        pending = None

        def emit_mm2(u):
            ntok, hbs, cb_t, w2f_l, b2t_l, slot0 = state[u]
            for t in range(2 * ntok):
                po = ppo.tile([128, D], F32, tag="po")
                for f in range(FTILES):
                    for dc in range(NDC):
                        nc.tensor.matmul(
                            po[:, dc * DCH:(dc + 1) * DCH],
                            lhsT=hbs[f][:, t * 128:(t + 1) * 128],
                            rhs=w2f_l[f][:, dc * DCH:(dc + 1) * DCH],
                            start=(f == 0),
                            stop=(f == FTILES - 1),
                        )
                yt = ypool.tile([128, D], F32, tag="yt")
                nc.vector.tensor_tensor(yt[:], po[:], b2t_l[:], ALU.add)
                yt2 = ypool.tile([128, D], F32, tag="yt2")
                nc.scalar.activation(yt2[:], yt[:], AF.Identity,
                                     scale=cb_t[:, t:t + 1])
                sl, tt = divmod(t, 2)
                nc.scalar.dma_start(
                    y[slot0 + sl, tt * 128:(tt + 1) * 128, :], yt2[:])

        gcur = -1
        w1f = w2f = b1t = b2t = None
        for ui, (g, slot0, ntok) in enumerate(units):
            if g != gcur:
                gcur = g
                b1t = bpool.tile([128, FTILES], F32, tag="b1")
                nc.gpsimd.dma_start(b1t[:], b1[g])
                w1f = []
                for f in range(FTILES):
                    wt = w1pool.tile([128, KD, 128], F32R, tag="w1f")
                    nc.gpsimd.dma_start(wt[:], w1[g, f])
                    w1f.append(wt)
                w2f = []
                for f in range(FTILES):
                    wt = w2pool.tile([128, D], F32R, tag="w2f")
                    nc.gpsimd.dma_start(wt[:], w2[g, f])
                    w2f.append(wt)
                b2t = bpool.tile([128, D], F32, tag="b2")
                nc.gpsimd.dma_start(b2t[:], b2[g])

            ntok_el = ntok * NT
            xtk = []
            for k in range(KD):
                xk = xpool.tile([128, 2 * NT], F32R, tag="xtk")
                nc.sync.dma_start(xk[:], xt[ui, k])
                xtk.append(xk)
            cb_t = cpool.tile([128, 4], F32, tag="cb")
            nc.sync.dma_start(cb_t[:], comb[ui])

            hbs = []
            for f in range(FTILES):
                ph = pph.tile([128, 2 * NT], F32, tag="ph")
                for k in range(KD):
                    nc.tensor.matmul(
                        ph[:, :ntok_el],
                        lhsT=w1f[f][:, k, :],
                        rhs=xtk[k][:, :ntok_el],
                        start=(k == 0),
                        stop=(k == KD - 1),
                    )
                hb = hpool.tile([128, 2 * NT], F32R, tag="hb")
                nc.scalar.activation(hb[:, :ntok_el], ph[:, :ntok_el],
                                     AF.Gelu, bias=b1t[:, f:f + 1])
                hbs.append(hb)
            state[ui] = (ntok, hbs, cb_t, list(w2f), b2t, slot0)
            if pending is not None:
                emit_mm2(pending)
                del state[pending]
            pending = ui
        if pending is not None:
            emit_mm2(pending)
    nc.compile()
    return nc


# ------------------------------------------------------------------ main --
LAST_EXEC_NS = None


def kernel(x, expert_weights, gate_w, gate_b, w1, b1, w2, b2, _trace=False):
    global LAST_EXEC_NS
    from concourse.bass_utils import run_bass_kernel_spmd

    x = np.asarray(x, np.float32)
    expert_weights = np.asarray(expert_weights, np.float32)
    gate_w = np.asarray(gate_w, np.float32)
    gate_b = np.asarray(gate_b, np.float32)
    w1 = np.asarray(w1, np.float32)
    b1 = np.asarray(b1, np.float32)
    w2 = np.asarray(w2, np.float32)
    b2 = np.asarray(b2, np.float32)

    combine, load_loss = _route(x, expert_weights, gate_w, gate_b)

    # token lists per expert, padded to NT multiples (pad index = -1)
    pad_idx_e, chunk_counts = [], []
    for e in range(E):
        idx = np.nonzero(combine[:, e] > 0)[0].astype(np.int64)
        nch = -(-len(idx) // NT) if len(idx) else 0
        padded = np.full(nch * NT, -1, np.int64)
        padded[:len(idx)] = idx
        pad_idx_e.append(padded)
        chunk_counts.append(nch)

    caps, assign = _plan(chunk_counts)
    G, TOTCH = len(caps), sum(caps)
    gbase = np.cumsum([0] + caps)
    units = _units(caps)
    NU = len(units)
    # map global chunk slot -> (unit index, half)
    slot_unit = {}
    for ui, (g, slot0, ntok) in enumerate(units):
        for s in range(ntok):
            slot_unit[slot0 + s] = (ui, s)

    # ---- host-side input assembly --------------------------------------
    xf = x.reshape(T, D)
    X_T = np.ascontiguousarray(xf.T)  # (D, T)

    w1c, w2c, b1c = {}, {}, {}

    def w1_block(e, q):
        key = (e, q)
        if key not in w1c:
            blk = w1[e][:, q * FQ:(q + 1) * FQ]            # (D, FQ)
            # [f, p, k, m] = W1q[k*128+p, f*128+m]
            w1c[key] = np.ascontiguousarray(
                blk.reshape(KD, 128, FTILES, 128).transpose(2, 1, 0, 3))
        return w1c[key]

    def w2_block(e, q):
        key = (e, q)
        if key not in w2c:
            blk = w2[e][q * FQ:(q + 1) * FQ, :]            # (FQ, D)
            # [f, p, :] = W2q[f*128+p, :]
            w2c[key] = np.ascontiguousarray(blk.reshape(FTILES, 128, D))
        return w2c[key]

    def b1_block(e, q):
        key = (e, q)
        if key not in b1c:
            b1c[key] = np.ascontiguousarray(
                b1[e][q * FQ:(q + 1) * FQ].reshape(FTILES, 128).T)
        return b1c[key]

    in_maps = []
    piece_meta = []  # per core: list of (slot0, n, toks)
    for core in range(NCORES):
        xt_in = np.zeros((NU, KD, 128, 2 * NT), np.float32)
        cb_in = np.zeros((NU, 128, 4), np.float32)
        w1_in = np.zeros((G, FTILES, 128, KD, 128), np.float32)
        w2_in = np.zeros((G, FTILES, 128, D), np.float32)
        b1_in = np.zeros((G, 128, FTILES), np.float32)
        b2_in = np.zeros((G, 128, D), np.float32)
        metas = []
        for g in range(G):
            piece = assign[g][core]
            if piece is None:
                continue
            e, q, off, n = piece
            w1_in[g] = w1_block(e, q)
            w2_in[g] = w2_block(e, q)
            b1_in[g] = b1_block(e, q)
            if q == 0:
                b2_in[g] = b2[e][None, :]
            toks = pad_idx_e[e][off * NT:(off + n) * NT]
            valid = toks >= 0
            seg = np.zeros((D, n * NT), np.float32)
            seg[:, valid] = X_T[:, toks[valid]]
            seg = seg.reshape(KD, 128, n, NT)        # [k, p, chunk, t]
            cvals = np.zeros(n * NT, np.float32)
            cvals[valid] = combine[toks[valid], e]
            cvals = cvals.reshape(n, 2, 128)         # [chunk, ttile, p]
            s0 = gbase[g]
            for j in range(n):
                ui, half = slot_unit[s0 + j]
                xt_in[ui, :, :, half * NT:(half + 1) * NT] = seg[:, :, j, :]
                cb_in[ui, :, half * 2:half * 2 + 2] = cvals[j].T
            metas.append((s0, n, toks))
        piece_meta.append(metas)
        in_maps.append({"xt": xt_in, "w1": w1_in, "w2": w2_in,
                        "b1": b1_in, "b2": b2_in, "comb": cb_in})

    nc = _build(caps)
    res = run_bass_kernel_spmd(nc, in_maps, list(range(NCORES)), trace=_trace)
    LAST_EXEC_NS = res.exec_time_ns

    # ---- gather / unshard ----------------------------------------------
    out = np.zeros((T, D), np.float32)
    for core in range(NCORES):
        yc = res.results[core]["y"]  # (TOTCH, NT, D), token-major
        for (s0, n, toks) in piece_meta[core]:
            rows = yc[s0:s0 + n].reshape(n * NT, D)
            valid = toks >= 0
            out[toks[valid]] += rows[valid]
    return out.reshape(B, S, D), load_loss
